# revision 1
# baseline (speedup 1.0000x reference)
"""BilinearAttention Trainium2 kernel: out = dropout(softmax(x W x^T)) @ x.

Problem: B=8, N=2048, D=1024, fp32. Data-parallel over batch across the 8
NeuronCores (one batch row per core); within a core the N x N score matrix
is built in 128-query-row blocks (flash-style row blocking).

Per-core dataflow (all matmuls on TensorE, fp32-class precision):
  pass0:  yT[e,n] = sum_d W[d,e] xT[d,n]        (x^T resident in SBUF)
  per 128-row block ns:
    s[ns,:] = yT[:,ns]^T @ xT   -> PSUM [128, 2048]
    rowmax (DVE), exp(s - max) + rowsum via one ACT pass
    dropout mask multiply (DVE), PE-transpose of the prob block,
    out[ns,:] = attnT^T @ x * (1/(0.9*rowsum))  (PSUM-accumulated PV)

Precision modes:
  "A": scores in f32r (TF32-like 11-bit), attn/PV bf16  (~6e-3 absmax rel)
  "B": scores via 3-term bf16 hi/lo split, attn/PV f32r (~2e-4 absmax rel)

The dropout mask is a fixed function of shape and jax key 42 (threefry is
platform-deterministic), so it is precomputed on host CPU and passed in.
"""
import os
import sys

sys.path.insert(0, "/opt/trn_rl_repo")

import numpy as np
import ml_dtypes

import concourse.bass as bass
import concourse.mybir as mybir
import concourse.tile as tile
from concourse import bacc
from concourse.bass import ts, ds
from concourse.masks import make_identity

F32 = mybir.dt.float32
F32R = mybir.dt.float32r
BF16 = mybir.dt.bfloat16
AF = mybir.ActivationFunctionType
AX = mybir.AxisListType

MODE = os.environ.get("BASS_ATTN_MODE", "B")
N_CORES = 8
DROPOUT_P = 0.1
DROPOUT_KEY = 42


def build(mode=MODE, N=2048, D=1024, repeat=1):
    assert N % 512 == 0 and D % 512 == 0
    ET = D // 128     # contraction slices
    NS = N // 128     # query row blocks
    MT = N // 128     # key/value tiles
    MCH = N // 512    # 512-wide column chunks of the score row-block
    DCH = D // 512
    INV_KEEP = float(1.0 / (1.0 - DROPOUT_P))

    at_dt = BF16 if mode == "A" else F32R   # attn / attnT / xv dtype

    nc = bacc.Bacc("TRN2", target_bir_lowering=False, debug=False)

    d_xt = nc.dram_tensor("xt", (D, N), F32R, kind="ExternalInput")
    d_xv = nc.dram_tensor("xv", (N, D), at_dt, kind="ExternalInput")
    d_w = nc.dram_tensor("w", (D, D), F32R, kind="ExternalInput")
    d_mask = nc.dram_tensor("mask", (N, N), at_dt, kind="ExternalInput")
    d_out = nc.dram_tensor("out", (N, D), F32, kind="ExternalOutput")
    if mode == "B":
        d_xth = nc.dram_tensor("xth", (D, N), BF16, kind="ExternalInput")
        d_xtl = nc.dram_tensor("xtl", (D, N), BF16, kind="ExternalInput")
        d_wh = nc.dram_tensor("wh", (D, D), BF16, kind="ExternalInput")
        d_wl = nc.dram_tensor("wl", (D, D), BF16, kind="ExternalInput")

    with tile.TileContext(nc) as tc:
        with tc.tile_pool(name="dram", bufs=1, space="DRAM") as dram, \
             tc.tile_pool(name="big", bufs=1) as big, \
             tc.tile_pool(name="wrk", bufs=2) as wrk, \
             tc.tile_pool(name="wrk1", bufs=1) as wrk1, \
             tc.tile_pool(name="st", bufs=3) as st, \
             tc.tile_pool(name="pss", bufs=1, space="PSUM") as pss, \
             tc.tile_pool(name="pst", bufs=2, space="PSUM") as pst, \
             tc.tile_pool(name="pso", bufs=2, space="PSUM") as pso:

            # ---------- resident inputs ----------
            if mode == "A":
                xt_sb = big.tile([128, ET * N], F32R, tag="xt")
                for e in range(ET):
                    nc.sync.dma_start(xt_sb[:, ts(e, N)], d_xt[ts(e, 128), :])
                xt_slice = lambda e, mch: xt_sb[:, ds(e * N + mch * 512, 512)]
            else:
                xth_sb = big.tile([128, ET * N], BF16, tag="xth")
                xtl_sb = big.tile([128, ET * N], BF16, tag="xtl")
                for e in range(ET):
                    nc.sync.dma_start(xth_sb[:, ts(e, N)], d_xth[ts(e, 128), :])
                    nc.sync.dma_start(xtl_sb[:, ts(e, N)], d_xtl[ts(e, 128), :])

            xv_sb = big.tile([128, MT * D], at_dt, tag="xv")
            for m in range(MT):
                nc.sync.dma_start(xv_sb[:, ts(m, D)], d_xv[ts(m, 128), :])

            if at_dt == F32R:
                ident_f = big.tile([128, 128], F32, tag="identf")
                make_identity(nc, ident_f[:])
                ident = big.tile([128, 128], F32R, tag="ident")
                nc.vector.tensor_copy(ident[:], ident_f[:])
            else:
                ident = big.tile([128, 128], at_dt, tag="ident")
                make_identity(nc, ident[:])

            if mode == "A":
                yt_dr = dram.tile([D, N], F32R, tag="yt")
            else:
                yth_dr = dram.tile([D, N], BF16, tag="yth")
                ytl_dr = dram.tile([D, N], BF16, tag="ytl")

            def body(_i=None, unroll=1):
                # ---------- pass 0: yT[e,n] = sum_d W[d,e] xT[d,n] ----------
                for e in range(ET):
                    if mode == "A":
                        wblk = wrk.tile([128, ET * 128], F32R, tag="wblk")
                        nc.sync.dma_start(
                            wblk[:].rearrange("p (a q) -> p a q", q=128),
                            d_w[:, ts(e, 128)].rearrange("(a p) q -> p a q", p=128))
                        ytile = wrk.tile([128, N], F32R, tag="ytile")
                    else:
                        whblk = wrk1.tile([128, ET * 128], BF16, tag="whblk")
                        wlblk = wrk1.tile([128, ET * 128], BF16, tag="wlblk")
                        nc.sync.dma_start(
                            whblk[:].rearrange("p (a q) -> p a q", q=128),
                            d_wh[:, ts(e, 128)].rearrange("(a p) q -> p a q", p=128))
                        nc.sync.dma_start(
                            wlblk[:].rearrange("p (a q) -> p a q", q=128),
                            d_wl[:, ts(e, 128)].rearrange("(a p) q -> p a q", p=128))
                        ythile = wrk1.tile([128, N], BF16, tag="ythile")
                        ytlile = wrk1.tile([128, N], BF16, tag="ytlile")
                    ps0 = pss.tile([128, N], F32, tag="ps_s")
                    for nch in range(MCH):
                        ps = ps0[:, ts(nch, 512)]
                        if mode == "A":
                            for d_i in range(ET):
                                nc.tensor.matmul(ps, wblk[:, ts(d_i, 128)],
                                                 xt_slice(d_i, nch),
                                                 start=(d_i == 0),
                                                 stop=(d_i == ET - 1))
                            nc.scalar.copy(ytile[:, ts(nch, 512)], ps)
                        else:
                            first = True
                            for d_i in range(ET):
                                xh = xth_sb[:, ds(d_i * N + nch * 512, 512)]
                                xl = xtl_sb[:, ds(d_i * N + nch * 512, 512)]
                                for (wv, xx) in ((whblk, xh), (whblk, xl),
                                                 (wlblk, xh)):
                                    nc.tensor.matmul(
                                        ps, wv[:, ts(d_i, 128)], xx,
                                        start=first,
                                        stop=(d_i == ET - 1 and wv is wlblk))
                                    first = False
                            nc.scalar.copy(ythile[:, ts(nch, 512)], ps)
                            nc.vector.tensor_sub(ytlile[:, ts(nch, 512)], ps,
                                                 ythile[:, ts(nch, 512)])
                    if mode == "A":
                        nc.sync.dma_start(yt_dr[ts(e, 128), :], ytile[:])
                    else:
                        nc.sync.dma_start(yth_dr[ts(e, 128), :], ythile[:])
                        nc.sync.dma_start(ytl_dr[ts(e, 128), :], ytlile[:])

                # ---------- main loop over query row blocks ----------
                for ns in range(NS):
                    if mode == "A":
                        ysb = wrk.tile([128, ET * 128], F32R, tag="ysb")
                        nc.sync.dma_start(
                            ysb[:].rearrange("p (a q) -> p a q", q=128),
                            yt_dr[:, ts(ns, 128)].rearrange(
                                "(a p) q -> p a q", p=128))
                    else:
                        yhsb = wrk.tile([128, ET * 128], BF16, tag="yhsb")
                        ylsb = wrk.tile([128, ET * 128], BF16, tag="ylsb")
                        nc.sync.dma_start(
                            yhsb[:].rearrange("p (a q) -> p a q", q=128),
                            yth_dr[:, ts(ns, 128)].rearrange(
                                "(a p) q -> p a q", p=128))
                        nc.sync.dma_start(
                            ylsb[:].rearrange("p (a q) -> p a q", q=128),
                            ytl_dr[:, ts(ns, 128)].rearrange(
                                "(a p) q -> p a q", p=128))

                    masksb = wrk.tile([128, N], at_dt, tag="masksb")
                    nc.sync.dma_start(masksb[:], d_mask[ts(ns, 128), :])

                    ps_s = pss.tile([128, N], F32, tag="ps_s")
                    for mch in range(MCH):
                        if mode == "A":
                            for e in range(ET):
                                nc.tensor.matmul(ps_s[:, ts(mch, 512)],
                                                 ysb[:, ts(e, 128)],
                                                 xt_slice(e, mch),
                                                 start=(e == 0),
                                                 stop=(e == ET - 1))
                        else:
                            first = True
                            for e in range(ET):
                                xh = xth_sb[:, ds(e * N + mch * 512, 512)]
                                xl = xtl_sb[:, ds(e * N + mch * 512, 512)]
                                for (yv, xx) in ((yhsb, xh), (yhsb, xl),
                                                 (ylsb, xh)):
                                    nc.tensor.matmul(
                                        ps_s[:, ts(mch, 512)],
                                        yv[:, ts(e, 128)], xx,
                                        start=first,
                                        stop=(e == ET - 1 and yv is ylsb))
                                    first = False

                    negmax = st.tile([128, 1], F32, tag="negmax")
                    nc.vector.reduce_max(negmax[:], ps_s[:], axis=AX.X,
                                         negate=True)

                    attn = wrk.tile([128, N], at_dt, tag="attn")
                    sumexp = st.tile([128, 1], F32, tag="sumexp")
                    nc.scalar.activation(attn[:], ps_s[:], AF.Exp,
                                         bias=negmax[:], scale=1.0,
                                         accum_out=sumexp[:])
                    inv = st.tile([128, 1], F32, tag="inv")
                    nc.vector.reciprocal(inv[:], sumexp[:])
                    nc.vector.tensor_scalar_mul(inv[:], inv[:], INV_KEEP)

                    # dropout zeroing (denominator is pre-mask, so after exp)
                    nc.vector.tensor_mul(attn[:], attn[:], masksb[:])

                    attnT = (wrk if mode == "A" else wrk1).tile(
                        [128, MT * 128], at_dt, tag="attnT")
                    for mt in range(MT):
                        pt = pst.tile([128, 128], at_dt, tag="pt")
                        nc.tensor.transpose(pt[:], attn[:, ts(mt, 128)],
                                            ident[:])
                        nc.scalar.copy(attnT[:, ts(mt, 128)], pt[:])

                    outsb = wrk.tile([128, D], F32, tag="outsb")
                    for dch in range(DCH):
                        po = pso.tile([128, 512], F32, tag="po")
                        for mt in range(MT):
                            nc.tensor.matmul(
                                po[:], attnT[:, ts(mt, 128)],
                                xv_sb[:, ds(mt * D + dch * 512, 512)],
                                start=(mt == 0), stop=(mt == MT - 1))
                        nc.scalar.activation(outsb[:, ts(dch, 512)], po[:],
                                             AF.Copy, bias=0.0, scale=inv[:])
                    nc.sync.dma_start(d_out[ts(ns, 128), :], outsb[:])

            if repeat == 1:
                body()
            else:
                with tc.For_i(0, repeat, 1):
                    body()

    nc.compile()
    return nc


def _split_hi_lo(a):
    hi = a.astype(ml_dtypes.bfloat16)
    lo = (a - hi.astype(np.float32)).astype(ml_dtypes.bfloat16)
    return hi, lo


def make_in_map(mode, xb, w, keepb):
    """xb [N,D] f32, w [D,D] f32, keepb [N,N] {0,1}."""
    xt = np.ascontiguousarray(xb.T)
    m = {"xt": xt, "w": w}
    if mode == "A":
        m["mask"] = keepb.astype(ml_dtypes.bfloat16)
        m["xv"] = xb.astype(ml_dtypes.bfloat16)
    else:
        m["mask"] = keepb.astype(np.float32)
        m["xv"] = xb.copy()
        m["xth"], m["xtl"] = _split_hi_lo(xt)
        m["wh"], m["wl"] = _split_hi_lo(w)
    return m


def dropout_keep_mask(B, N):
    """Reproduce jax.random.bernoulli(key(42), 0.9, (B,N,N)) on host CPU.

    Threefry is bit-exact across backends, so this matches the reference."""
    import jax
    cpu = jax.devices("cpu")[0]
    with jax.default_device(cpu):
        keep = jax.random.bernoulli(
            jax.random.key(DROPOUT_KEY), 1.0 - DROPOUT_P, (B, N, N))
        return np.asarray(keep).astype(np.float32)


_NC_CACHE = {}


def _get_nc(mode, N, D):
    key = (mode, N, D)
    if key not in _NC_CACHE:
        _NC_CACHE[key] = build(mode, N, D)
    return _NC_CACHE[key]


def kernel(x, weight):
    from concourse.bass_utils import run_bass_kernel_spmd
    x = np.asarray(x, dtype=np.float32)
    w = np.asarray(weight, dtype=np.float32)
    B, N, D = x.shape
    assert B == N_CORES
    keep = dropout_keep_mask(B, N)
    nc = _get_nc(MODE, N, D)
    in_maps = [make_in_map(MODE, x[b], w, keep[b]) for b in range(B)]
    res = run_bass_kernel_spmd(nc, in_maps, core_ids=list(range(N_CORES)))
    return np.stack([res.results[b]["out"] for b in range(B)]).astype(np.float32)
